# revision 13
# baseline (speedup 1.0000x reference)
"""ContentConcatAttention Trainium2 kernel (8 NeuronCores, data-parallel over batch).

reference:
    cat   = concat([context, broadcast(x)], -1)        # [B, T, DC+DI]
    h     = tanh(cat @ W1)                             # [B, T, DH]
    score = h @ W2                                     # [B, T]
    attn  = softmax(score, axis=1)
    c     = einsum('bt,btd->bd', attn, context)        # [B, DC]
    returns (c, attn)

Kernel algebra: cat @ W1 == context @ W1c + x @ W1x  (W1c = W1[:DC], W1x = W1[DC:]),
so the broadcast/concat is never materialized and the flops are halved.
Each core handles B/8 = 4 batches. All matmuls run in float32r (full-rate fp32).
"""
import sys
import numpy as np

if '/opt/trn_rl_repo' not in sys.path:
    sys.path.insert(0, '/opt/trn_rl_repo')

import concourse.bass as bass
import concourse.mybir as mybir
import concourse.tile as tile
from concourse.tile import add_dep_helper

B, T, DC, DI, DH = 32, 1024, 512, 512, 1024
NCORES = 8
BL = B // NCORES            # batches per core = 4
NTOK = BL * T               # tokens per core = 4096
CH = 512                    # token chunk (moving free dim)
NCHUNK = NTOK // CH         # 8 chunks per core, 2 per batch
KC = DC // 128              # 4 contraction tiles for context features
KX = DI // 128              # 4 contraction tiles for x features
MH = DH // 128              # 8 hidden tiles
KT = T // 128               # 8 token k-tiles per batch (weighted sum)

F32 = mybir.dt.float32
F32R = mybir.dt.float32r
AF = mybir.ActivationFunctionType

_compiled = {}


# Instruction classes whose walrus lowering has only one sync-wait slot.
_SPLIT_OPS = None  # all instruction classes have a single wait slot

def _split_multi_waits(nc, max_waits=1):
    """The public neuronxcc walrus supports a single sync-wait slot on some
    instruction formats (fused-weight-load Matmult, Drain). Hoist extra waits
    into standalone single-wait EventSemaphore instructions placed
    immediately before, on the same engine (engines execute waits in
    dispatch order, so this is equivalent)."""
    cnt = 0
    for f in nc.m.functions:
        for bb in f.blocks:
            insts = bb.instructions
            if not any(i.sync_info and len(i.sync_info.on_wait) > max_waits
                       for i in insts):
                continue
            new = []
            for inst in insts:
                si = inst.sync_info
                if si is not None and len(si.on_wait) > max_waits:
                    waits = list(si.on_wait)
                    for w in waits[max_waits:]:
                        cnt += 1
                        new.append(mybir.InstEventSemaphore(
                            name=f"hoistw-{cnt}", engine=inst.engine,
                            bass_nofuse=True,
                            sync_info=mybir.SyncInfo(on_wait=[w], on_update=[])))
                    inst.sync_info = mybir.SyncInfo(
                        on_wait=waits[:max_waits], on_update=list(si.on_update))
                new.append(inst)
            insts[:] = new
    return cnt


def _build():
    nc = bass.Bass()

    ctxT = nc.dram_tensor("ctxT", [DC, NTOK], F32R, kind="ExternalInput")
    ctx = nc.dram_tensor("ctx", [NTOK, DC], F32R, kind="ExternalInput")
    w1c = nc.dram_tensor("w1c", [DC, DH], F32R, kind="ExternalInput")
    w1x = nc.dram_tensor("w1x", [DI, DH], F32R, kind="ExternalInput")
    xT = nc.dram_tensor("xT", [DI, BL], F32R, kind="ExternalInput")
    w2r = nc.dram_tensor("w2r", [DH, 128], F32R, kind="ExternalInput")
    ident = nc.dram_tensor("ident", [128, 128], F32, kind="ExternalInput")
    c_out = nc.dram_tensor("c_out", [BL, DC], F32, kind="ExternalOutput")
    attn_out = nc.dram_tensor("attn_out", [BL, T], F32, kind="ExternalOutput")
    import os
    dbg = os.environ.get("CCA_DEBUG") == "1"
    if dbg:
        xbT_dbg = nc.dram_tensor("xbT_dbg", [128, MH, BL], F32, kind="ExternalOutput")
        sc_dbg = nc.dram_tensor("sc_dbg", [BL, T], F32, kind="ExternalOutput")
        h_dbg = nc.dram_tensor("h_dbg", [128, T], F32, kind="ExternalOutput")

    with tile.TileContext(nc) as tc:
        with (
            tc.tile_pool(name="big", bufs=1) as big,
            tc.tile_pool(name="hp", bufs=3) as hp,
            tc.tile_pool(name="rows", bufs=1) as rows,
            tc.tile_pool(name="php", bufs=3, space="PSUM") as php,
            tc.tile_pool(name="scp", bufs=2, space="PSUM") as scp,
            tc.tile_pool(name="msp", bufs=3, space="PSUM") as msp,
        ):
            # ---- persistent SBUF tensors ----
            ctxT_sb = big.tile([128, KC, NTOK], F32R, tag="ctxT")
            ctx_sb = big.tile([128, NTOK // 128, DC], F32R, tag="ctx")
            w1c_sb = big.tile([128, KC, DH], F32R, tag="w1c")
            w1x_sb = big.tile([128, KX, DH], F32R, tag="w1x")
            xT_sb = big.tile([128, KX, BL], F32R, tag="xT")
            w2r_sb = big.tile([128, MH, 128], F32R, tag="w2r")
            id_sb = big.tile([128, 128], F32, tag="ident")
            xbT_sb = big.tile([128, MH, BL], F32, tag="xbT")
            attnT_sb = big.tile([128, KT, BL], F32R, tag="attnT")

            scoreF = [rows.tile([128, T], F32, tag=f"scoreF{b}",
                                name=f"scoreF{b}") for b in range(BL)]
            expR = rows.tile([128, T], F32, tag="expR")
            attnR = rows.tile([128, T], F32, tag="attnR")
            xb_sb = rows.tile([BL, DH], F32, tag="xb")
            cR = rows.tile([BL, DC], F32, tag="cR")
            mneg = rows.tile([128, 1], F32, tag="mneg")
            ssum = rows.tile([128, 1], F32, tag="ssum")
            rsum = rows.tile([128, 1], F32, tag="rsum")

            # ---- input DMAs ----
            ctxT_r = ctxT.rearrange("(k p) n -> p k n", p=128)
            ctx_r = ctx.rearrange("(n p) d -> p n d", p=128)
            nc.sync.dma_start(w1c_sb[:], w1c.rearrange("(k p) m -> p k m", p=128))
            nc.sync.dma_start(w1x_sb[:], w1x.rearrange("(k p) m -> p k m", p=128))
            nc.sync.dma_start(xT_sb[:], xT.rearrange("(k p) b -> p k b", p=128))
            nc.sync.dma_start(w2r_sb[:], w2r.rearrange("(m p) b -> p m b", p=128))
            nc.sync.dma_start(id_sb[:], ident[:])
            # context loads, chunked for pipeline startup
            for c in range(NCHUNK):
                for k in range(KC):
                    nc.sync.dma_start(ctxT_sb[:, k, c * CH:(c + 1) * CH],
                                      ctxT_r[:, k, c * CH:(c + 1) * CH])
            for b in range(BL):
                nc.sync.dma_start(
                    ctx_sb[:, b * KT:(b + 1) * KT, :],
                    ctx_r[:, b * KT:(b + 1) * KT, :])

            # ---- deferred emission machinery (controls PE stream order) ----
            group = 0
            deferred = []  # (fire_at_group, fn)

            def fire(g):
                due = [d for d in deferred if d[0] <= g]
                deferred[:] = [d for d in deferred if d[0] > g]
                for _, fn in due:
                    fn()

            def defer(delay, fn):
                deferred.append((group + delay, fn))

            # ---- prologue: xb = x @ W1x  ([BL, DH]), then transpose to xbT ----
            for piece in range(2):
                xb_ps = msp.tile([BL, 512], F32, tag="m")
                for k in range(KX):
                    nc.tensor.matmul(xb_ps[:], xT_sb[:, k, :],
                                     w1x_sb[:, k, piece * 512:(piece + 1) * 512],
                                     start=(k == 0), stop=(k == KX - 1))
                nc.vector.tensor_copy(xb_sb[:, piece * 512:(piece + 1) * 512], xb_ps[:])

            xbT_copies = []
            for m in range(MH):
                tp = msp.tile([128, BL], F32, tag="m", name=f"tpx{m}")
                nc.tensor.transpose(tp[:], xb_sb[:, m * 128:(m + 1) * 128],
                                    id_sb[0:BL, 0:BL])
                xbT_copies.append(nc.vector.tensor_copy(xbT_sb[:, m, :], tp[:]))

            # ---- batch tail: softmax, attn transpose, weighted sum ----
            def emit_softmax(b):
                P = 32 * b
                red = nc.vector.tensor_reduce(out=mneg[P:P + 1, :],
                                              in_=scoreF[b][P:P + 1, :],
                                              op=mybir.AluOpType.max,
                                              axis=mybir.AxisListType.X, negate=True)
                ex = nc.scalar.activation(expR[P:P + 1, :], scoreF[b][P:P + 1, :],
                                          AF.Exp, bias=mneg[P:P + 1, :], scale=1.0,
                                          accum_out=ssum[P:P + 1, :])
                add_dep_helper(ex.ins, red.ins, sync=True,
                               reason="exp bias reads -max")
                rc = nc.vector.reciprocal(rsum[P:P + 1, :], ssum[P:P + 1, :])
                add_dep_helper(rc.ins, ex.ins, sync=True,
                               reason="reciprocal reads exp accum")
                nc.vector.tensor_scalar_mul(attnR[P:P + 1, :], expR[P:P + 1, :],
                                            rsum[P:P + 1, :])
                nc.sync.dma_start(attn_out[b:b + 1, :], attnR[P:P + 1, :])

            def emit_attnT(b):
                for k in range(KT):
                    tp = msp.tile([128, 97], F32, tag="m")
                    nc.tensor.transpose(tp[:], attnR[0:97, k * 128:(k + 1) * 128],
                                        id_sb[0:97, 0:97])
                    nc.vector.tensor_copy(attnT_sb[:, k, b:b + 1],
                                          tp[:, 32 * b:32 * b + 1])

            def emit_weighted(b):
                c_ps = msp.tile([1, DC], F32, tag="m")
                for k in range(KT):
                    nc.tensor.matmul(c_ps[:], attnT_sb[:, k, b:b + 1],
                                     ctx_sb[:, b * KT + k, :],
                                     start=(k == 0), stop=(k == KT - 1),
                                     skip_group_check=True)
                nc.vector.tensor_copy(cR[0:1, :], c_ps[:])
                nc.sync.dma_start(c_out[b:b + 1, :], cR[0:1, :])

            # ---- main loop over token chunks ----
            for c in range(NCHUNK):
                b, half = divmod(c, 2)
                sc_ps = scp.tile([128, CH], F32, tag="scps")

                def make_score(sc_ps, h_sb, m):
                    def f():
                        nc.tensor.matmul(sc_ps[:], w2r_sb[:, m, :], h_sb[:],
                                         start=(m == 0), stop=(m == MH - 1),
                                         skip_group_check=True)
                    return f

                def make_score_done(sc_ps, b, half):
                    def f():
                        nc.vector.tensor_copy(
                            scoreF[b][:, half * CH:(half + 1) * CH],
                            sc_ps[:])
                        if half == 1:
                            emit_softmax(b)
                    return f

                for m in range(MH):
                    ph = php.tile([128, CH], F32, tag="ph")
                    for k in range(KC):
                        nc.tensor.matmul(ph[:], w1c_sb[:, k, m * 128:(m + 1) * 128],
                                         ctxT_sb[:, k, c * CH:(c + 1) * CH],
                                         start=(k == 0), stop=(k == KC - 1))
                    h_sb = hp.tile([128, CH], F32R, tag="h")
                    th = nc.scalar.activation(h_sb[:], ph[:], AF.Tanh,
                                              bias=xbT_sb[:, m, b:b + 1], scale=1.0)
                    add_dep_helper(th.ins, xbT_copies[m].ins, sync=True,
                                   reason="tanh bias reads xbT")
                    if dbg and c == 0 and m == 0:
                        nc.sync.dma_start(h_dbg[:, 0:CH], h_sb[:].bitcast(F32))
                    if dbg and c == 1 and m == 0:
                        nc.sync.dma_start(h_dbg[:, CH:2 * CH], h_sb[:].bitcast(F32))
                    defer(1, make_score(sc_ps, h_sb, m))
                    group += 1
                    fire(group)

                # after score(7) of this chunk fires (group c*8+8), copy + softmax
                defer(1, make_score_done(sc_ps, b, half))
                if half == 1:
                    defer(4, (lambda b=b: emit_attnT(b)))
                    defer(5, (lambda b=b: emit_weighted(b)))

            # flush remaining deferred work in firing order
            for g, fn in sorted(deferred, key=lambda d: d[0]):
                fn()
            deferred.clear()
            if dbg:
                nc.sync.dma_start(xbT_dbg[:], xbT_sb[:])
                for b in range(BL):
                    nc.sync.dma_start(sc_dbg[b:b + 1, :],
                                      scoreF[b][32 * b:32 * b + 1, :])

    _split_multi_waits(nc)
    return nc


def _get_nc():
    if "nc" not in _compiled:
        _compiled["nc"] = _build()
    return _compiled["nc"]


def _host_prep(x, w, context, W1, W2):
    x = np.asarray(x, dtype=np.float32)
    context = np.asarray(context, dtype=np.float32)
    W1 = np.asarray(W1, dtype=np.float32)
    W2 = np.asarray(W2, dtype=np.float32)
    w1c = np.ascontiguousarray(W1[:DC])               # [DC, DH]
    w1x = np.ascontiguousarray(W1[DC:])               # [DI, DH]
    w2r = np.ascontiguousarray(
        np.repeat(W2.reshape(DH, 1), 128, axis=1))    # [DH, 128]
    ident = np.eye(128, dtype=np.float32)
    in_maps = []
    for i in range(NCORES):
        ctx_i = np.ascontiguousarray(
            context[i * BL:(i + 1) * BL].reshape(NTOK, DC))
        in_maps.append({
            "ctx": ctx_i,
            "ctxT": np.ascontiguousarray(ctx_i.T),
            "w1c": w1c,
            "w1x": w1x,
            "xT": np.ascontiguousarray(x[i * BL:(i + 1) * BL].T),
            "w2r": w2r,
            "ident": ident,
        })
    return in_maps


def run(inputs, trace=False, tmpdir=None):
    from concourse.bass_utils import run_bass_kernel_spmd
    if trace:
        _register_ntff_hook()
    nc = _get_nc()
    in_maps = _host_prep(**inputs)
    res = run_bass_kernel_spmd(nc, in_maps, core_ids=list(range(NCORES)),
                               trace=trace, tmpdir=tmpdir)
    c = np.concatenate([r["c_out"] for r in res.results], axis=0)
    attn = np.concatenate([r["attn_out"] for r in res.results], axis=0)
    return (c, attn), res


def _register_ntff_hook():
    """The stub antenv package lacks axon_hooks; register it so
    run_bass_kernel_spmd(trace=True) can collect NTFF profiles."""
    import types
    import antenv
    if getattr(antenv, "axon_hooks", None) is not None:
        return
    m = types.ModuleType('antenv.axon_hooks')
    m._hook = None
    m.set_axon_ntff_profile_hook = lambda h: setattr(m, '_hook', h)
    m.get_axon_ntff_profile_hook = lambda: m._hook
    sys.modules['antenv.axon_hooks'] = m
    antenv.axon_hooks = m
    try:
        from trn_agent_boot.trn_boot import _ntff_profile_via_ctypes
        m._hook = _ntff_profile_via_ctypes('/opt/axon/libaxon_pjrt.so')
    except Exception:
        m._hook = None


def kernel(x, w, context, W1, W2):
    (c, attn), _ = run(dict(x=x, w=w, context=context, W1=W1, W2=W2))
    return (c, attn)


# revision 14
# speedup vs baseline: 1.1092x; 1.1092x over previous
"""ContentConcatAttention Trainium2 kernel (8 NeuronCores, data-parallel over batch).

reference:
    cat   = concat([context, broadcast(x)], -1)        # [B, T, DC+DI]
    h     = tanh(cat @ W1)                             # [B, T, DH]
    score = h @ W2                                     # [B, T]
    attn  = softmax(score, axis=1)
    c     = einsum('bt,btd->bd', attn, context)        # [B, DC]
    returns (c, attn)

Kernel algebra: cat @ W1 == context @ W1c + x @ W1x  (W1c = W1[:DC], W1x = W1[DC:]),
so the broadcast/concat is never materialized and the flops are halved.
Each core handles B/8 = 4 batches. All matmuls run in float32r (full-rate fp32).
"""
import sys
import numpy as np

if '/opt/trn_rl_repo' not in sys.path:
    sys.path.insert(0, '/opt/trn_rl_repo')

import concourse.bass as bass
import concourse.mybir as mybir
import concourse.tile as tile
from concourse.tile import add_dep_helper

B, T, DC, DI, DH = 32, 1024, 512, 512, 1024
NCORES = 8
BL = B // NCORES            # batches per core = 4
NTOK = BL * T               # tokens per core = 4096
CH = 512                    # token chunk (moving free dim)
NCHUNK = NTOK // CH         # 8 chunks per core, 2 per batch
KC = DC // 128              # 4 contraction tiles for context features
KX = DI // 128              # 4 contraction tiles for x features
MH = DH // 128              # 8 hidden tiles
KT = T // 128               # 8 token k-tiles per batch (weighted sum)

F32 = mybir.dt.float32
F32R = mybir.dt.float32r
BF16 = mybir.dt.bfloat16
AF = mybir.ActivationFunctionType

_compiled = {}


# Instruction classes whose walrus lowering has only one sync-wait slot.
_SPLIT_OPS = None  # all instruction classes have a single wait slot

def _split_multi_waits(nc, max_waits=1):
    """The public neuronxcc walrus supports a single sync-wait slot on some
    instruction formats (fused-weight-load Matmult, Drain). Hoist extra waits
    into standalone single-wait EventSemaphore instructions placed
    immediately before, on the same engine (engines execute waits in
    dispatch order, so this is equivalent)."""
    cnt = 0
    for f in nc.m.functions:
        for bb in f.blocks:
            insts = bb.instructions
            if not any(i.sync_info and len(i.sync_info.on_wait) > max_waits
                       for i in insts):
                continue
            new = []
            for inst in insts:
                si = inst.sync_info
                if si is not None and len(si.on_wait) > max_waits:
                    waits = list(si.on_wait)
                    for w in waits[max_waits:]:
                        cnt += 1
                        new.append(mybir.InstEventSemaphore(
                            name=f"hoistw-{cnt}", engine=inst.engine,
                            bass_nofuse=True,
                            sync_info=mybir.SyncInfo(on_wait=[w], on_update=[])))
                    inst.sync_info = mybir.SyncInfo(
                        on_wait=waits[:max_waits], on_update=list(si.on_update))
                new.append(inst)
            insts[:] = new
    return cnt


def _build():
    nc = bass.Bass()

    ctxT = nc.dram_tensor("ctxT", [DC, NTOK], BF16, kind="ExternalInput")
    ctx = nc.dram_tensor("ctx", [NTOK, DC], BF16, kind="ExternalInput")
    w1c = nc.dram_tensor("w1c", [DC, DH], BF16, kind="ExternalInput")
    w1x = nc.dram_tensor("w1x", [DI, DH], BF16, kind="ExternalInput")
    xT = nc.dram_tensor("xT", [DI, BL], BF16, kind="ExternalInput")
    w2r = nc.dram_tensor("w2r", [DH, 128], BF16, kind="ExternalInput")
    ident = nc.dram_tensor("ident", [128, 128], F32, kind="ExternalInput")
    c_out = nc.dram_tensor("c_out", [BL, DC], F32, kind="ExternalOutput")
    attn_out = nc.dram_tensor("attn_out", [BL, T], F32, kind="ExternalOutput")
    import os
    dbg = os.environ.get("CCA_DEBUG") == "1"
    if dbg:
        xbT_dbg = nc.dram_tensor("xbT_dbg", [128, MH, BL], F32, kind="ExternalOutput")
        sc_dbg = nc.dram_tensor("sc_dbg", [BL, T], F32, kind="ExternalOutput")
        h_dbg = nc.dram_tensor("h_dbg", [128, T], F32, kind="ExternalOutput")

    with tile.TileContext(nc) as tc:
        with (
            tc.tile_pool(name="big", bufs=1) as big,
            tc.tile_pool(name="hp", bufs=3) as hp,
            tc.tile_pool(name="rows", bufs=1) as rows,
            tc.tile_pool(name="php", bufs=3, space="PSUM") as php,
            tc.tile_pool(name="scp", bufs=2, space="PSUM") as scp,
            tc.tile_pool(name="msp", bufs=3, space="PSUM") as msp,
        ):
            # ---- persistent SBUF tensors ----
            ctxT_sb = big.tile([128, KC, NTOK], BF16, tag="ctxT")
            ctx_sb = big.tile([128, NTOK // 128, DC], BF16, tag="ctx")
            w1c_sb = big.tile([128, KC, DH], BF16, tag="w1c")
            w1x_sb = big.tile([128, KX, DH], BF16, tag="w1x")
            xT_sb = big.tile([128, KX, BL], BF16, tag="xT")
            w2r_sb = big.tile([128, MH, 128], BF16, tag="w2r")
            id_sb = big.tile([128, 128], F32, tag="ident")
            xbT_sb = big.tile([128, MH, BL], F32, tag="xbT")
            attnT_sb = big.tile([128, KT, BL], BF16, tag="attnT")

            scoreF = [rows.tile([128, T], F32, tag=f"scoreF{b}",
                                name=f"scoreF{b}") for b in range(BL)]
            expR = rows.tile([128, T], F32, tag="expR")
            attnR = rows.tile([128, T], F32, tag="attnR")
            xb_sb = rows.tile([BL, DH], F32, tag="xb")
            cR = rows.tile([BL, DC], F32, tag="cR")
            mneg = rows.tile([128, 1], F32, tag="mneg")
            ssum = rows.tile([128, 1], F32, tag="ssum")
            rsum = rows.tile([128, 1], F32, tag="rsum")

            # ---- input DMAs ----
            ctxT_r = ctxT.rearrange("(k p) n -> p k n", p=128)
            ctx_r = ctx.rearrange("(n p) d -> p n d", p=128)
            nc.sync.dma_start(w1c_sb[:], w1c.rearrange("(k p) m -> p k m", p=128))
            nc.sync.dma_start(w1x_sb[:], w1x.rearrange("(k p) m -> p k m", p=128))
            nc.sync.dma_start(xT_sb[:], xT.rearrange("(k p) b -> p k b", p=128))
            nc.sync.dma_start(w2r_sb[:], w2r.rearrange("(m p) b -> p m b", p=128))
            nc.sync.dma_start(id_sb[:], ident[:])
            # context loads, chunked for pipeline startup
            for c in range(NCHUNK):
                for k in range(KC):
                    nc.sync.dma_start(ctxT_sb[:, k, c * CH:(c + 1) * CH],
                                      ctxT_r[:, k, c * CH:(c + 1) * CH])
            for b in range(BL):
                nc.sync.dma_start(
                    ctx_sb[:, b * KT:(b + 1) * KT, :],
                    ctx_r[:, b * KT:(b + 1) * KT, :])

            # ---- deferred emission machinery (controls PE stream order) ----
            group = 0
            deferred = []  # (fire_at_group, fn)

            def fire(g):
                due = [d for d in deferred if d[0] <= g]
                deferred[:] = [d for d in deferred if d[0] > g]
                for _, fn in due:
                    fn()

            def defer(delay, fn):
                deferred.append((group + delay, fn))

            # ---- prologue: xb = x @ W1x  ([BL, DH]), then transpose to xbT ----
            for piece in range(2):
                xb_ps = msp.tile([BL, 512], F32, tag="m")
                for k in range(KX):
                    nc.tensor.matmul(xb_ps[:], xT_sb[:, k, :],
                                     w1x_sb[:, k, piece * 512:(piece + 1) * 512],
                                     start=(k == 0), stop=(k == KX - 1))
                nc.vector.tensor_copy(xb_sb[:, piece * 512:(piece + 1) * 512], xb_ps[:])

            xbT_copies = []
            for m in range(MH):
                tp = msp.tile([128, BL], F32, tag="m", name=f"tpx{m}")
                nc.tensor.transpose(tp[:], xb_sb[:, m * 128:(m + 1) * 128],
                                    id_sb[0:BL, 0:BL])
                xbT_copies.append(nc.vector.tensor_copy(xbT_sb[:, m, :], tp[:]))

            # ---- batch tail: softmax, attn transpose, weighted sum ----
            def emit_softmax(b):
                P = 32 * b
                red = nc.vector.tensor_reduce(out=mneg[P:P + 1, :],
                                              in_=scoreF[b][P:P + 1, :],
                                              op=mybir.AluOpType.max,
                                              axis=mybir.AxisListType.X, negate=True)
                ex = nc.scalar.activation(expR[P:P + 1, :], scoreF[b][P:P + 1, :],
                                          AF.Exp, bias=mneg[P:P + 1, :], scale=1.0,
                                          accum_out=ssum[P:P + 1, :])
                add_dep_helper(ex.ins, red.ins, sync=True,
                               reason="exp bias reads -max")
                rc = nc.vector.reciprocal(rsum[P:P + 1, :], ssum[P:P + 1, :])
                add_dep_helper(rc.ins, ex.ins, sync=True,
                               reason="reciprocal reads exp accum")
                nc.vector.tensor_scalar_mul(attnR[P:P + 1, :], expR[P:P + 1, :],
                                            rsum[P:P + 1, :])
                nc.sync.dma_start(attn_out[b:b + 1, :], attnR[P:P + 1, :])

            def emit_attnT(b):
                for k in range(KT):
                    tp = msp.tile([128, 97], F32, tag="m")
                    nc.tensor.transpose(tp[:], attnR[0:97, k * 128:(k + 1) * 128],
                                        id_sb[0:97, 0:97])
                    nc.vector.tensor_copy(attnT_sb[:, k, b:b + 1],
                                          tp[:, 32 * b:32 * b + 1])

            def emit_weighted(b):
                c_ps = msp.tile([1, DC], F32, tag="m")
                for k in range(KT):
                    nc.tensor.matmul(c_ps[:], attnT_sb[:, k, b:b + 1],
                                     ctx_sb[:, b * KT + k, :],
                                     start=(k == 0), stop=(k == KT - 1),
                                     skip_group_check=True)
                nc.vector.tensor_copy(cR[0:1, :], c_ps[:])
                nc.sync.dma_start(c_out[b:b + 1, :], cR[0:1, :])

            # ---- main loop over token chunks ----
            for c in range(NCHUNK):
                b, half = divmod(c, 2)
                sc_ps = scp.tile([128, CH], F32, tag="scps")

                def make_score(sc_ps, h_sb, m):
                    def f():
                        nc.tensor.matmul(sc_ps[:], w2r_sb[:, m, :], h_sb[:],
                                         start=(m == 0), stop=(m == MH - 1),
                                         skip_group_check=True)
                    return f

                def make_score_done(sc_ps, b, half):
                    def f():
                        nc.vector.tensor_copy(
                            scoreF[b][:, half * CH:(half + 1) * CH],
                            sc_ps[:])
                        if half == 1:
                            emit_softmax(b)
                    return f

                for m in range(MH):
                    ph = php.tile([128, CH], F32, tag="ph")
                    for k in range(KC):
                        nc.tensor.matmul(ph[:], w1c_sb[:, k, m * 128:(m + 1) * 128],
                                         ctxT_sb[:, k, c * CH:(c + 1) * CH],
                                         start=(k == 0), stop=(k == KC - 1))
                    h_sb = hp.tile([128, CH], BF16, tag="h")
                    th = nc.scalar.activation(h_sb[:], ph[:], AF.Tanh,
                                              bias=xbT_sb[:, m, b:b + 1], scale=1.0)
                    add_dep_helper(th.ins, xbT_copies[m].ins, sync=True,
                                   reason="tanh bias reads xbT")
                    if dbg and c == 0 and m == 0:
                        nc.sync.dma_start(h_dbg[:, 0:CH], h_sb[:].bitcast(F32))
                    if dbg and c == 1 and m == 0:
                        nc.sync.dma_start(h_dbg[:, CH:2 * CH], h_sb[:].bitcast(F32))
                    defer(1, make_score(sc_ps, h_sb, m))
                    group += 1
                    fire(group)

                # after score(7) of this chunk fires (group c*8+8), copy + softmax
                defer(1, make_score_done(sc_ps, b, half))
                if half == 1:
                    defer(4, (lambda b=b: emit_attnT(b)))
                    defer(5, (lambda b=b: emit_weighted(b)))

            # flush remaining deferred work in firing order
            for g, fn in sorted(deferred, key=lambda d: d[0]):
                fn()
            deferred.clear()
            if dbg:
                nc.sync.dma_start(xbT_dbg[:], xbT_sb[:])
                for b in range(BL):
                    nc.sync.dma_start(sc_dbg[b:b + 1, :],
                                      scoreF[b][32 * b:32 * b + 1, :])

    _split_multi_waits(nc)
    return nc


def _get_nc():
    if "nc" not in _compiled:
        _compiled["nc"] = _build()
    return _compiled["nc"]


def _host_prep(x, w, context, W1, W2):
    import ml_dtypes
    bf16 = ml_dtypes.bfloat16
    x = np.asarray(x, dtype=np.float32)
    context = np.asarray(context, dtype=np.float32).astype(bf16)
    W1 = np.asarray(W1, dtype=np.float32)
    W2 = np.asarray(W2, dtype=np.float32)
    w1c = np.ascontiguousarray(W1[:DC].astype(bf16))  # [DC, DH]
    w1x = np.ascontiguousarray(W1[DC:].astype(bf16))  # [DI, DH]
    w2r = np.ascontiguousarray(
        np.repeat(W2.reshape(DH, 1), 128, axis=1).astype(bf16))  # [DH, 128]
    ident = np.eye(128, dtype=np.float32)
    in_maps = []
    for i in range(NCORES):
        ctx_i = np.ascontiguousarray(
            context[i * BL:(i + 1) * BL].reshape(NTOK, DC))
        in_maps.append({
            "ctx": ctx_i,
            "ctxT": np.ascontiguousarray(ctx_i.T),
            "w1c": w1c,
            "w1x": w1x,
            "xT": np.ascontiguousarray(x[i * BL:(i + 1) * BL].T.astype(bf16)),
            "w2r": w2r,
            "ident": ident,
        })
    return in_maps


def run(inputs, trace=False, tmpdir=None):
    from concourse.bass_utils import run_bass_kernel_spmd
    if trace:
        _register_ntff_hook()
    nc = _get_nc()
    in_maps = _host_prep(**inputs)
    res = run_bass_kernel_spmd(nc, in_maps, core_ids=list(range(NCORES)),
                               trace=trace, tmpdir=tmpdir)
    c = np.concatenate([r["c_out"] for r in res.results], axis=0)
    attn = np.concatenate([r["attn_out"] for r in res.results], axis=0)
    return (c, attn), res


def _register_ntff_hook():
    """The stub antenv package lacks axon_hooks; register it so
    run_bass_kernel_spmd(trace=True) can collect NTFF profiles."""
    import types
    import antenv
    if getattr(antenv, "axon_hooks", None) is not None:
        return
    m = types.ModuleType('antenv.axon_hooks')
    m._hook = None
    m.set_axon_ntff_profile_hook = lambda h: setattr(m, '_hook', h)
    m.get_axon_ntff_profile_hook = lambda: m._hook
    sys.modules['antenv.axon_hooks'] = m
    antenv.axon_hooks = m
    try:
        from trn_agent_boot.trn_boot import _ntff_profile_via_ctypes
        m._hook = _ntff_profile_via_ctypes('/opt/axon/libaxon_pjrt.so')
    except Exception:
        m._hook = None


def kernel(x, w, context, W1, W2):
    (c, attn), _ = run(dict(x=x, w=w, context=context, W1=W1, W2=W2))
    return (c, attn)


# revision 15
# speedup vs baseline: 1.1110x; 1.0016x over previous
"""ContentConcatAttention Trainium2 kernel (8 NeuronCores, data-parallel over batch).

reference:
    cat   = concat([context, broadcast(x)], -1)        # [B, T, DC+DI]
    h     = tanh(cat @ W1)                             # [B, T, DH]
    score = h @ W2                                     # [B, T]
    attn  = softmax(score, axis=1)
    c     = einsum('bt,btd->bd', attn, context)        # [B, DC]
    returns (c, attn)

Kernel algebra: cat @ W1 == context @ W1c + x @ W1x  (W1c = W1[:DC], W1x = W1[DC:]),
so the broadcast/concat is never materialized and the flops are halved.
Each core handles B/8 = 4 batches. All matmuls run in float32r (full-rate fp32).
"""
import sys
import numpy as np

if '/opt/trn_rl_repo' not in sys.path:
    sys.path.insert(0, '/opt/trn_rl_repo')

import concourse.bass as bass
import concourse.mybir as mybir
import concourse.tile as tile
from concourse.tile import add_dep_helper

B, T, DC, DI, DH = 32, 1024, 512, 512, 1024
NCORES = 8
BL = B // NCORES            # batches per core = 4
NTOK = BL * T               # tokens per core = 4096
CH = 512                    # token chunk (moving free dim)
NCHUNK = NTOK // CH         # 8 chunks per core, 2 per batch
KC = DC // 128              # 4 contraction tiles for context features
KX = DI // 128              # 4 contraction tiles for x features
MH = DH // 128              # 8 hidden tiles
KT = T // 128               # 8 token k-tiles per batch (weighted sum)

F32 = mybir.dt.float32
F32R = mybir.dt.float32r
BF16 = mybir.dt.bfloat16
AF = mybir.ActivationFunctionType

_compiled = {}


# Instruction classes whose walrus lowering has only one sync-wait slot.
_SPLIT_OPS = None  # all instruction classes have a single wait slot

def _split_multi_waits(nc, max_waits=1):
    """The public neuronxcc walrus supports a single sync-wait slot on some
    instruction formats (fused-weight-load Matmult, Drain). Hoist extra waits
    into standalone single-wait EventSemaphore instructions placed
    immediately before, on the same engine (engines execute waits in
    dispatch order, so this is equivalent)."""
    cnt = 0
    for f in nc.m.functions:
        for bb in f.blocks:
            insts = bb.instructions
            if not any(i.sync_info and len(i.sync_info.on_wait) > max_waits
                       for i in insts):
                continue
            new = []
            for inst in insts:
                si = inst.sync_info
                if si is not None and len(si.on_wait) > max_waits:
                    waits = list(si.on_wait)
                    for w in waits[max_waits:]:
                        cnt += 1
                        new.append(mybir.InstEventSemaphore(
                            name=f"hoistw-{cnt}", engine=inst.engine,
                            bass_nofuse=True,
                            sync_info=mybir.SyncInfo(on_wait=[w], on_update=[])))
                    inst.sync_info = mybir.SyncInfo(
                        on_wait=waits[:max_waits], on_update=list(si.on_update))
                new.append(inst)
            insts[:] = new
    return cnt


def _build():
    nc = bass.Bass()

    ctxT = nc.dram_tensor("ctxT", [DC, NTOK], BF16, kind="ExternalInput")
    ctx = nc.dram_tensor("ctx", [NTOK, DC], BF16, kind="ExternalInput")
    w1c = nc.dram_tensor("w1c", [DC, DH], BF16, kind="ExternalInput")
    w1x = nc.dram_tensor("w1x", [DI, DH], BF16, kind="ExternalInput")
    xT = nc.dram_tensor("xT", [DI, BL], BF16, kind="ExternalInput")
    w2r = nc.dram_tensor("w2r", [DH, 128], BF16, kind="ExternalInput")
    ident = nc.dram_tensor("ident", [128, 128], F32, kind="ExternalInput")
    c_out = nc.dram_tensor("c_out", [BL, DC], F32, kind="ExternalOutput")
    attn_out = nc.dram_tensor("attn_out", [BL, T], F32, kind="ExternalOutput")
    import os
    dbg = os.environ.get("CCA_DEBUG") == "1"
    if dbg:
        xbT_dbg = nc.dram_tensor("xbT_dbg", [128, MH, BL], F32, kind="ExternalOutput")
        sc_dbg = nc.dram_tensor("sc_dbg", [BL, T], F32, kind="ExternalOutput")
        h_dbg = nc.dram_tensor("h_dbg", [128, T], F32, kind="ExternalOutput")

    with tile.TileContext(nc) as tc:
        with (
            tc.tile_pool(name="big", bufs=1) as big,
            tc.tile_pool(name="hp", bufs=4) as hp,
            tc.tile_pool(name="rows", bufs=1) as rows,
            tc.tile_pool(name="php", bufs=3, space="PSUM") as php,
            tc.tile_pool(name="scp", bufs=2, space="PSUM") as scp,
            tc.tile_pool(name="msp", bufs=3, space="PSUM") as msp,
        ):
            # ---- persistent SBUF tensors ----
            ctxT_sb = big.tile([128, KC, NTOK], BF16, tag="ctxT")
            ctx_sb = big.tile([128, NTOK // 128, DC], BF16, tag="ctx")
            w1c_sb = big.tile([128, KC, DH], BF16, tag="w1c")
            w1x_sb = big.tile([128, KX, DH], BF16, tag="w1x")
            xT_sb = big.tile([128, KX, BL], BF16, tag="xT")
            w2r_sb = big.tile([128, MH, 128], BF16, tag="w2r")
            id_sb = big.tile([128, 128], F32, tag="ident")
            xbT_sb = big.tile([128, MH, BL], F32, tag="xbT")
            attnT_sb = big.tile([128, KT, BL], BF16, tag="attnT")

            scoreF = [rows.tile([128, T], F32, tag=f"scoreF{b}",
                                name=f"scoreF{b}") for b in range(BL)]
            expR = rows.tile([128, T], F32, tag="expR")
            attnR = rows.tile([128, T], F32, tag="attnR")
            xb_sb = rows.tile([BL, DH], F32, tag="xb")
            cR = rows.tile([BL, DC], F32, tag="cR")
            mneg = rows.tile([128, 1], F32, tag="mneg")
            ssum = rows.tile([128, 1], F32, tag="ssum")
            rsum = rows.tile([128, 1], F32, tag="rsum")

            # ---- input DMAs ----
            ctxT_r = ctxT.rearrange("(k p) n -> p k n", p=128)
            ctx_r = ctx.rearrange("(n p) d -> p n d", p=128)
            nc.sync.dma_start(w1c_sb[:], w1c.rearrange("(k p) m -> p k m", p=128))
            nc.sync.dma_start(w1x_sb[:], w1x.rearrange("(k p) m -> p k m", p=128))
            nc.sync.dma_start(xT_sb[:], xT.rearrange("(k p) b -> p k b", p=128))
            nc.sync.dma_start(w2r_sb[:], w2r.rearrange("(m p) b -> p m b", p=128))
            nc.sync.dma_start(id_sb[:], ident[:])
            # context loads, chunked for pipeline startup
            for c in range(NCHUNK):
                for k in range(KC):
                    nc.sync.dma_start(ctxT_sb[:, k, c * CH:(c + 1) * CH],
                                      ctxT_r[:, k, c * CH:(c + 1) * CH])


            # ---- deferred emission machinery (controls PE stream order) ----
            group = 0
            deferred = []  # (fire_at_group, fn)

            def fire(g):
                due = [d for d in deferred if d[0] <= g]
                deferred[:] = [d for d in deferred if d[0] > g]
                for _, fn in due:
                    fn()

            def defer(delay, fn):
                deferred.append((group + delay, fn))

            # ---- prologue: xb = x @ W1x  ([BL, DH]), then transpose to xbT ----
            for piece in range(2):
                xb_ps = msp.tile([BL, 512], F32, tag="m")
                for k in range(KX):
                    nc.tensor.matmul(xb_ps[:], xT_sb[:, k, :],
                                     w1x_sb[:, k, piece * 512:(piece + 1) * 512],
                                     start=(k == 0), stop=(k == KX - 1))
                nc.vector.tensor_copy(xb_sb[:, piece * 512:(piece + 1) * 512], xb_ps[:])

            xbT_copies = []
            for m in range(MH):
                tp = msp.tile([128, BL], F32, tag="m", name=f"tpx{m}")
                nc.tensor.transpose(tp[:], xb_sb[:, m * 128:(m + 1) * 128],
                                    id_sb[0:BL, 0:BL])
                xbT_copies.append(nc.vector.tensor_copy(xbT_sb[:, m, :], tp[:]))

            # ---- batch tail: softmax, attn transpose, weighted sum ----
            def emit_softmax(b):
                P = 32 * b
                ex = nc.scalar.activation(expR[P:P + 1, :], scoreF[b][P:P + 1, :],
                                          AF.Exp, scale=1.0,
                                          accum_out=ssum[P:P + 1, :])
                rc = nc.vector.reciprocal(rsum[P:P + 1, :], ssum[P:P + 1, :])
                add_dep_helper(rc.ins, ex.ins, sync=True,
                               reason="reciprocal reads exp accum")
                nc.vector.tensor_scalar_mul(attnR[P:P + 1, :], expR[P:P + 1, :],
                                            rsum[P:P + 1, :])
                nc.sync.dma_start(attn_out[b:b + 1, :], attnR[P:P + 1, :])

            def emit_attnT(b):
                for k in range(KT):
                    tp = msp.tile([128, 97], F32, tag="m")
                    nc.tensor.transpose(tp[:], attnR[0:97, k * 128:(k + 1) * 128],
                                        id_sb[0:97, 0:97])
                    nc.vector.tensor_copy(attnT_sb[:, k, b:b + 1],
                                          tp[:, 32 * b:32 * b + 1])

            def emit_weighted(b):
                c_ps = msp.tile([1, DC], F32, tag="m")
                for k in range(KT):
                    nc.tensor.matmul(c_ps[:], attnT_sb[:, k, b:b + 1],
                                     ctx_sb[:, b * KT + k, :],
                                     start=(k == 0), stop=(k == KT - 1),
                                     skip_group_check=True)
                nc.vector.tensor_copy(cR[0:1, :], c_ps[:])
                nc.sync.dma_start(c_out[b:b + 1, :], cR[0:1, :])

            # ---- main loop over token chunks ----
            for c in range(NCHUNK):
                b, half = divmod(c, 2)
                if half == 0:
                    nc.sync.dma_start(
                        ctx_sb[:, b * KT:(b + 1) * KT, :],
                        ctx_r[:, b * KT:(b + 1) * KT, :])
                sc_ps = scp.tile([128, CH], F32, tag="scps")

                def make_score(sc_ps, h_sb, m):
                    def f():
                        nc.tensor.matmul(sc_ps[:], w2r_sb[:, m, :], h_sb[:],
                                         start=(m == 0), stop=(m == MH - 1),
                                         skip_group_check=True)
                    return f

                def make_score_done(sc_ps, b, half):
                    def f():
                        nc.vector.tensor_copy(
                            scoreF[b][:, half * CH:(half + 1) * CH],
                            sc_ps[:])
                        if half == 1:
                            emit_softmax(b)
                    return f

                for m in range(MH):
                    ph = php.tile([128, CH], F32, tag="ph")
                    for k in range(KC):
                        nc.tensor.matmul(ph[:], w1c_sb[:, k, m * 128:(m + 1) * 128],
                                         ctxT_sb[:, k, c * CH:(c + 1) * CH],
                                         start=(k == 0), stop=(k == KC - 1))
                    h_sb = hp.tile([128, CH], BF16, tag="h")
                    th = nc.scalar.activation(h_sb[:], ph[:], AF.Tanh,
                                              bias=xbT_sb[:, m, b:b + 1], scale=1.0)
                    add_dep_helper(th.ins, xbT_copies[m].ins, sync=True,
                                   reason="tanh bias reads xbT")
                    if dbg and c == 0 and m == 0:
                        nc.sync.dma_start(h_dbg[:, 0:CH], h_sb[:].bitcast(F32))
                    if dbg and c == 1 and m == 0:
                        nc.sync.dma_start(h_dbg[:, CH:2 * CH], h_sb[:].bitcast(F32))
                    defer(1, make_score(sc_ps, h_sb, m))
                    group += 1
                    fire(group)

                # fires after score(7) (which lands at group c*8+9)
                defer(2, make_score_done(sc_ps, b, half))
                if half == 1:
                    defer(4, (lambda b=b: emit_attnT(b)))
                    defer(5, (lambda b=b: emit_weighted(b)))

            # flush remaining deferred work in firing order
            for g, fn in sorted(deferred, key=lambda d: d[0]):
                fn()
            deferred.clear()
            if dbg:
                nc.sync.dma_start(xbT_dbg[:], xbT_sb[:])
                for b in range(BL):
                    nc.sync.dma_start(sc_dbg[b:b + 1, :],
                                      scoreF[b][32 * b:32 * b + 1, :])

    _split_multi_waits(nc)
    return nc


def _get_nc():
    if "nc" not in _compiled:
        _compiled["nc"] = _build()
    return _compiled["nc"]


def _host_prep(x, w, context, W1, W2):
    import ml_dtypes
    bf16 = ml_dtypes.bfloat16
    x = np.asarray(x, dtype=np.float32)
    context = np.asarray(context, dtype=np.float32).astype(bf16)
    W1 = np.asarray(W1, dtype=np.float32)
    W2 = np.asarray(W2, dtype=np.float32)
    w1c = np.ascontiguousarray(W1[:DC].astype(bf16))  # [DC, DH]
    w1x = np.ascontiguousarray(W1[DC:].astype(bf16))  # [DI, DH]
    w2r = np.ascontiguousarray(
        np.repeat(W2.reshape(DH, 1), 128, axis=1).astype(bf16))  # [DH, 128]
    ident = np.eye(128, dtype=np.float32)
    in_maps = []
    for i in range(NCORES):
        ctx_i = np.ascontiguousarray(
            context[i * BL:(i + 1) * BL].reshape(NTOK, DC))
        in_maps.append({
            "ctx": ctx_i,
            "ctxT": np.ascontiguousarray(ctx_i.T),
            "w1c": w1c,
            "w1x": w1x,
            "xT": np.ascontiguousarray(x[i * BL:(i + 1) * BL].T.astype(bf16)),
            "w2r": w2r,
            "ident": ident,
        })
    return in_maps


def run(inputs, trace=False, tmpdir=None):
    from concourse.bass_utils import run_bass_kernel_spmd
    if trace:
        _register_ntff_hook()
    nc = _get_nc()
    in_maps = _host_prep(**inputs)
    res = run_bass_kernel_spmd(nc, in_maps, core_ids=list(range(NCORES)),
                               trace=trace, tmpdir=tmpdir)
    c = np.concatenate([r["c_out"] for r in res.results], axis=0)
    attn = np.concatenate([r["attn_out"] for r in res.results], axis=0)
    return (c, attn), res


def _register_ntff_hook():
    """The stub antenv package lacks axon_hooks; register it so
    run_bass_kernel_spmd(trace=True) can collect NTFF profiles."""
    import types
    import antenv
    if getattr(antenv, "axon_hooks", None) is not None:
        return
    m = types.ModuleType('antenv.axon_hooks')
    m._hook = None
    m.set_axon_ntff_profile_hook = lambda h: setattr(m, '_hook', h)
    m.get_axon_ntff_profile_hook = lambda: m._hook
    sys.modules['antenv.axon_hooks'] = m
    antenv.axon_hooks = m
    try:
        from trn_agent_boot.trn_boot import _ntff_profile_via_ctypes
        m._hook = _ntff_profile_via_ctypes('/opt/axon/libaxon_pjrt.so')
    except Exception:
        m._hook = None


def kernel(x, w, context, W1, W2):
    (c, attn), _ = run(dict(x=x, w=w, context=context, W1=W1, W2=W2))
    return (c, attn)
